# revision 5
# baseline (speedup 1.0000x reference)
"""Trainium2 Bass kernel for nn_CliffordTransformLayer (Cl(4,1) sandwich product).

Computes, per row n:
    tmp[k] = sum_{i,j} v[n,i] * p[n,j] * T1[i,j,k]
    out[l] = sum_{k,i} tmp[k] * v[n,i] * T2[k,i,l]

Strategy (per NeuronCore, data-parallel over 8 cores):
  * Host packs v [N,16] -> vpack [128, N/8]   (partition (w,i) = row 8n+w, feature i)
    and p [N,5] -> ppack [40, N/8].
  * All constant-table contractions are block-diagonal I_8 (x) (16x16) matmul
    weights on the tensor engine; 8 rows share each 128-partition column.
  * Only two data*data ops per tile run on the vector engine:
      u = vpack * Prep      (the 80 v_i*p_j products per row)
      q = tmp * G           (the 80 tmp_k*G products per row), where
      G_l[k] = sum_i T2[k,i,l]*v_i is linear in v -> tensor engine.
  * Per-tile PE work: 5 p-broadcast selector MMs, 5 accumulating T1 MMs,
    5 G MMs, 5 accumulating reduction MMs.
"""

import numpy as np

import concourse.bass as bass
import concourse.mybir as mybir
import concourse.tile as tile
from concourse.bass_utils import run_bass_kernel_spmd

# ---------------------------------------------------------------------------
# Cl(4,1) blade algebra tables (pure numpy; mirrors the reference definition).
# ---------------------------------------------------------------------------
METRIC = (1, 1, 1, 1, -1)


def _pc(x):
    return bin(x).count("1")


def _gp(a, b):
    s = 0
    t = a >> 1
    while t:
        s += _pc(t & b)
        t >>= 1
    sign = -1 if (s & 1) else 1
    common = a & b
    for i in range(5):
        if common & (1 << i):
            sign *= METRIC[i]
    return a ^ b, sign


def _build_tables():
    EVEN = [m for m in range(32) if _pc(m) % 2 == 0]
    ODD = [m for m in range(32) if _pc(m) % 2 == 1]
    VEC = [1, 2, 4, 8, 16]
    ODD_IDX = {m: i for i, m in enumerate(ODD)}
    VEC_IDX = {m: i for i, m in enumerate(VEC)}

    def _rev_sign(m):
        g = _pc(m)
        return -1 if (g * (g - 1) // 2) % 2 else 1

    T1 = np.zeros((16, 5, 16), dtype=np.float32)
    for i, e in enumerate(EVEN):
        for j, v in enumerate(VEC):
            blade, s = _gp(e, v)
            T1[i, j, ODD_IDX[blade]] = s
    T2 = np.zeros((16, 16, 5), dtype=np.float32)
    for k, o in enumerate(ODD):
        for i, e in enumerate(EVEN):
            blade, s = _gp(o, e)
            if _pc(blade) == 1:
                T2[k, i, VEC_IDX[blade]] = s * _rev_sign(e)
    return T1, T2


T1, T2 = _build_tables()

# ---------------------------------------------------------------------------
# Problem/layout constants.
# ---------------------------------------------------------------------------
N_TOTAL = 2_097_152
N_CORES = 8
N_CORE = N_TOTAL // N_CORES          # 262144 rows per core
W = 8                                # rows packed per 128-partition column
NCOL = N_CORE // W                   # 32768 columns per core
C = 256                              # columns per tile
N_TILES = NCOL // C                  # 128 tiles per core
DT = mybir.dt.float32
FP = np.float32


def _build_weights():
    """Block-diagonal weight matrices, already laid out for SBUF tiles."""
    # W1[j]: [128,128], W1[w*16+i, w*16+k] = T1[i,j,k]
    w1 = np.zeros((5, 128, 128), dtype=FP)
    # W2[l]: [128,128], W2[w*16+i, w*16+k] = T2[k,i,l]
    w2 = np.zeros((5, 128, 128), dtype=FP)
    # SelP[j]: [40,128], SelP[w*5+j, w*16+i] = 1  (broadcast p_j to 16 i-lanes)
    selp = np.zeros((5, 40, 128), dtype=FP)
    # WR[l]: [128,40], WR[w*16+k, w*5+l] = 1     (sum over k into out slot l)
    wr = np.zeros((5, 128, 40), dtype=FP)
    for w in range(W):
        r16 = slice(w * 16, w * 16 + 16)
        for j in range(5):
            w1[j][r16, r16] = T1[:, j, :]        # [i,k]
            w2[j][r16, r16] = T2[:, :, j].T      # [i,k] from T2[k,i,l=j]
            selp[j][w * 5 + j, r16] = 1.0
            wr[j][r16, w * 5 + j] = 1.0
    # Concatenate along free dim for single SBUF tiles.
    w1_sb = np.concatenate([w1[j] for j in range(5)], axis=1)      # [128, 640]
    w2_sb = np.concatenate([w2[l] for l in range(5)], axis=1)      # [128, 640]
    selp_sb = np.concatenate([selp[j] for j in range(5)], axis=1)  # [40, 640]
    wr_sb = np.concatenate([wr[l] for l in range(5)], axis=1)      # [128, 200]
    return w1_sb, w2_sb, selp_sb, wr_sb


W1_SB, W2_SB, SELP_SB, WR_SB = _build_weights()

_CACHED_NC = None


def _split_waits(nc, max_keep=1):
    """This container's walrus accepts only one sync-wait per instruction;
    hoist extra waits onto standalone EventSemaphore instructions."""
    n_split = 0
    for fn in nc.m.functions:
        for blk in fn.blocks:
            newlist = []
            for inst in blk.instructions:
                si = inst.sync_info
                if si is not None and si.on_wait and len(si.on_wait) > max_keep:
                    waits = list(si.on_wait)
                    for k, w in enumerate(waits[:-max_keep]):
                        ev = mybir.InstEventSemaphore(
                            name=f"{inst.name}-wsplit{k}",
                            engine=inst.engine,
                            ins=[], outs=[],
                            sync_info=mybir.SyncInfo(on_wait=[w], on_update=[]),
                        )
                        newlist.append(ev)
                        n_split += 1
                    inst.sync_info = mybir.SyncInfo(
                        on_wait=waits[-max_keep:],
                        on_update=list(si.on_update) if si.on_update else [])
                newlist.append(inst)
            blk.instructions = newlist
    return n_split


def _build_bass():
    global _CACHED_NC
    if _CACHED_NC is not None:
        return _CACHED_NC
    nc = bass.Bass("TRN2", target_bir_lowering=False, debug=False,
                   num_devices=N_CORES)

    vpack_d = nc.dram_tensor("vpack", [128, NCOL], DT, kind="ExternalInput")
    ppack_d = nc.dram_tensor("ppack", [40, NCOL], DT, kind="ExternalInput")
    w1_d = nc.dram_tensor("w1", [128, 640], DT, kind="ExternalInput")
    w2_d = nc.dram_tensor("w2", [128, 640], DT, kind="ExternalInput")
    selp_d = nc.dram_tensor("selp", [40, 640], DT, kind="ExternalInput")
    wr_d = nc.dram_tensor("wr", [128, 200], DT, kind="ExternalInput")
    out_d = nc.dram_tensor("outpack", [40, NCOL], DT, kind="ExternalOutput")

    with tile.TileContext(nc) as tc:
        with (
            tc.tile_pool(name="consts", bufs=1) as consts,
            tc.tile_pool(name="vin", bufs=3) as vin,
            tc.tile_pool(name="pin", bufs=3) as pin,
            tc.tile_pool(name="usb", bufs=2) as usb,
            tc.tile_pool(name="tsb", bufs=2) as tsb,
            tc.tile_pool(name="qsb", bufs=2) as qsb,
            tc.tile_pool(name="osb", bufs=3) as osb,
            tc.tile_pool(name="pg_ps", bufs=2, space="PSUM") as pg_ps,
            tc.tile_pool(name="tmp_ps", bufs=1, space="PSUM") as tmp_ps,
            tc.tile_pool(name="out_ps", bufs=1, space="PSUM") as out_ps,
        ):
            # Load constant weights once.
            w1_t = consts.tile([128, 640], DT, name="w1_t")
            w2_t = consts.tile([128, 640], DT, name="w2_t")
            selp_t = consts.tile([40, 640], DT, name="selp_t")
            wr_t = consts.tile([128, 200], DT, name="wr_t")
            nc.sync.dma_start(w1_t[:], w1_d[:, :])
            nc.sync.dma_start(w2_t[:], w2_d[:, :])
            nc.sync.dma_start(selp_t[:], selp_d[:, :])
            nc.sync.dma_start(wr_t[:], wr_d[:, :])

            for t in range(N_TILES):
                c0 = t * C
                vt = vin.tile([128, C], DT, name="vt")
                pt = pin.tile([40, C], DT, name="pt")
                nc.sync.dma_start(vt[:], vpack_d[:, c0:c0 + C])
                nc.sync.dma_start(pt[:], ppack_d[:, c0:c0 + C])

                # --- stage 1: Prep (p broadcast), u = v*p, tmp = T1-contract ---
                prep = pg_ps.tile([128, 5, C], DT, name="prep", tag="pg")
                for j in range(5):
                    nc.tensor.matmul(
                        prep[:, j, :],
                        selp_t[:, j * 128:(j + 1) * 128],
                        pt[:],
                        start=True, stop=True,
                    )
                u = usb.tile([128, 5, C], DT, name="u")
                nc.vector.tensor_tensor(
                    u[:],
                    prep[:],
                    vt[:].unsqueeze(1).broadcast_to([128, 5, C]),
                    mybir.AluOpType.mult,
                )
                tmp = tmp_ps.tile([128, C], DT, name="tmp")
                for j in range(5):
                    nc.tensor.matmul(
                        tmp[:],
                        w1_t[:, j * 128:(j + 1) * 128],
                        u[:, j, :],
                        start=(j == 0), stop=(j == 4),
                    )
                tmps = tsb.tile([128, C], DT, name="tmps")
                nc.scalar.copy(tmps[:], tmp[:])

                # --- stage 2: G = T2-contract(v), q = tmp*G, out = reduce ---
                g = pg_ps.tile([128, 5, C], DT, name="g", tag="pg")
                for l in range(5):
                    nc.tensor.matmul(
                        g[:, l, :],
                        w2_t[:, l * 128:(l + 1) * 128],
                        vt[:],
                        start=True, stop=True,
                    )
                q = qsb.tile([128, 5, C], DT, name="q")
                nc.vector.tensor_tensor(
                    q[:],
                    g[:],
                    tmps[:].unsqueeze(1).broadcast_to([128, 5, C]),
                    mybir.AluOpType.mult,
                )
                ops = out_ps.tile([40, C], DT, name="ops")
                for l in range(5):
                    nc.tensor.matmul(
                        ops[:],
                        wr_t[:, l * 40:(l + 1) * 40],
                        q[:, l, :],
                        start=(l == 0), stop=(l == 4),
                    )
                ot = osb.tile([40, C], DT, name="ot")
                nc.scalar.copy(ot[:], ops[:])
                nc.sync.dma_start(out_d[:, c0:c0 + C], ot[:])

    _split_waits(nc)
    _CACHED_NC = nc
    return nc


def _pack_inputs(versor, point):
    """Per-core packed inputs."""
    v = np.ascontiguousarray(versor, dtype=FP).reshape(N_CORES, N_CORE, 16)
    p = np.ascontiguousarray(point, dtype=FP).reshape(N_CORES, N_CORE, 5)
    in_maps = []
    for c in range(N_CORES):
        vp = v[c].reshape(NCOL, W, 16).transpose(1, 2, 0).reshape(128, NCOL)
        pp = p[c].reshape(NCOL, W, 5).transpose(1, 2, 0).reshape(40, NCOL)
        in_maps.append({
            "vpack": np.ascontiguousarray(vp),
            "ppack": np.ascontiguousarray(pp),
            "w1": W1_SB, "w2": W2_SB, "selp": SELP_SB, "wr": WR_SB,
        })
    return in_maps


def kernel(versor, point, trace=False):
    orig_dtype = point.dtype
    nc = _build_bass()
    in_maps = _pack_inputs(versor, point)
    res = run_bass_kernel_spmd(nc, in_maps, core_ids=list(range(N_CORES)),
                               trace=trace)
    outs = []
    for c in range(N_CORES):
        op = res.results[c]["outpack"]          # [40, NCOL]
        o = op.reshape(W, 5, NCOL).transpose(2, 0, 1).reshape(N_CORE, 5)
        outs.append(o)
    out = np.concatenate(outs, axis=0)
    kernel.last_results = res
    return out.astype(orig_dtype, copy=False)
